# revision 7
# baseline (speedup 1.0000x reference)
"""PointConvMiniSqueeze TRN2 kernel.

Math (fused form of the reference):
  For each batch b and point n:
    idx[n, 0:10] = indices of the 10 smallest pairwise squared distances
                   (self first; ties broken by lower index)
    G[n, k*3+cc] = pos[b, cc, idx[n, k]]                       (k=0..9, cc=0..2)
    out[b, p, n] = sum_j Wfused[p, j] * G[n, j] + bias_eff[p]

  Wfused = w2 @ W1eff, where W1eff folds the checkerboard re-gather (INDS),
  the 2x2-block squeeze ([a,d,b,c] channel order), and conv1's 2x2 kernel
  into a single [64, 30] matrix. bias_eff = w2 @ b1 + b2.

Device pipeline per batch (one NeuronCore per batch, 8 total):
  - S = -d2 via one K=5 matmul:  lhsT rows [x,y,z,nsq,1], rhs rows
    [2x,2y,2z,1,nsq], so S = 2*dot - sq_m - sq_n  (self-distance exactly 0).
  - top-10 per row on DVE (max8/max_index/match_replace/max8/max_index).
  - per-point gather of neighbor coords via indirect SWDGE DMA from
    pos_T [4096, 3] in DRAM.
  - PE transpose of the gathered [128, 30] tile, then K=30 matmul against
    WfusedT [30, 64] and a bias-add copy out.
"""

import numpy as np

import concourse.bass as bass
import concourse.tile as tile
from concourse import bacc, mybir
from concourse.bass_utils import run_bass_kernel_spmd
from concourse import masks

B, C, N = 8, 3, 4096
K = 10
TILE = 128
NTILES = N // TILE
CHUNK = 512
NCHUNK = N // CHUNK
F32 = mybir.dt.float32
U32 = mybir.dt.uint32

# squeeze group g in [a,d,b,c] order; (h,w) -> neighbor index k
_KMAP = (
    ((0, 2), (4, 6)),   # a: (0::2, 0::2) of the 4x4 -> INDS positions
    ((3, 5), (7, 9)),   # d
    ((1, 3), (5, 7)),   # b
    ((2, 4), (6, 8)),   # c
)


def _fuse_weights(w1, b1, w2, b2):
    w1 = w1.astype(np.float64)
    W1eff = np.zeros((64, 30), np.float64)
    for g in range(4):
        for cc in range(3):
            for h in range(2):
                for w in range(2):
                    k = _KMAP[g][h][w]
                    W1eff[:, k * 3 + cc] += w1[:, g * 3 + cc, h, w]
    Wfused = (w2.astype(np.float64) @ W1eff).astype(np.float32)       # [64, 30]
    bias_eff = (w2.astype(np.float64) @ b1.astype(np.float64)
                + b2.astype(np.float64)).astype(np.float32)           # [64]
    return Wfused, bias_eff


def _build_program():
    nc = bacc.Bacc(None, target_bir_lowering=False, debug=False)

    pos_d = nc.dram_tensor("pos", [C, N], F32, kind="ExternalInput")
    posT_d = nc.dram_tensor("pos_t", [N, C], F32, kind="ExternalInput")
    wf_d = nc.dram_tensor("wfused_t", [30, 64], F32, kind="ExternalInput")
    be_d = nc.dram_tensor("bias_eff", [64, 1], F32, kind="ExternalInput")
    out_d = nc.dram_tensor("out", [64, N], F32, kind="ExternalOutput")

    with tile.TileContext(nc) as tc:
        _emit(tc, nc, pos_d, posT_d, wf_d, be_d, out_d)
    nc.compile()
    return nc


def _emit(tc, nc, pos_d, posT_d, wf_d, be_d, out_d):
    from contextlib import ExitStack
    ctx = ExitStack()
    with ctx:
        const = ctx.enter_context(tc.tile_pool(name="const", bufs=1))
        spool = ctx.enter_context(tc.tile_pool(name="s", bufs=2))
        small = ctx.enter_context(tc.tile_pool(name="small", bufs=3))
        psum = ctx.enter_context(
            tc.tile_pool(name="psum", bufs=3, space="PSUM"))
        psum_t = ctx.enter_context(
            tc.tile_pool(name="psum_t", bufs=2, space="PSUM"))
        psum_o = ctx.enter_context(
            tc.tile_pool(name="psum_o", bufs=2, space="PSUM"))

        # ---- constants / prelims ----
        ident = const.tile([TILE, TILE], F32)
        masks.make_identity(nc, ident[:])

        wf = const.tile([30, 64], F32)
        nc.sync.dma_start(wf[:], wf_d[:])
        be = const.tile([64, 1], F32)
        nc.sync.dma_start(be[:], be_d[:])

        lhsT = const.tile([3, N], F32)
        nc.sync.dma_start(lhsT[:], pos_d[:])

        # nsq_row = -(x^2 + y^2 + z^2): exact negation of the reference's sq,
        # same add order, so S below is bit-exact -d2.
        negsq = const.tile([3, N], F32)
        nc.vector.scalar_tensor_tensor(
            negsq[:], lhsT[:], -1.0, lhsT[:],
            op0=mybir.AluOpType.mult, op1=mybir.AluOpType.mult)
        n1 = const.tile([1, N], F32)
        n2 = const.tile([1, N], F32)
        nc.sync.dma_start(n1[:], negsq[1:2, :])
        nc.sync.dma_start(n2[:], negsq[2:3, :])
        t01 = const.tile([1, N], F32)
        nsq_row = const.tile([1, N], F32)
        nc.vector.tensor_add(t01[:], negsq[0:1, :], n1[:])
        nc.vector.tensor_add(nsq_row[:], t01[:], n2[:])

        # per-tile query columns: nsq_cols[p, t] = nsq_row[0, t*128+p]
        nsq_cols = const.tile([TILE, NTILES], F32)
        for t in range(NTILES):
            nc.sync.dma_start(nsq_cols[:, t:t + 1],
                              nsq_row[0:1, bass.ts(t, TILE)])
        # candidate row broadcast to all partitions
        nsq_bcast = const.tile([TILE, N], F32)
        nc.gpsimd.partition_broadcast(nsq_bcast[:], nsq_row[:])

        # ---- main loop over 32 row-tiles of 128 points ----
        # S = fl(fl(2*dot - sq_q) - sq_c) == -d2 bit-exact vs the reference,
        # so top-k selection and rank order match jax.lax.top_k exactly.
        for t in range(NTILES):
            s = spool.tile([TILE, N], F32)
            for c in range(NCHUNK):
                ps = psum.tile([TILE, CHUNK], F32)
                nc.tensor.matmul(
                    ps[:],
                    lhsT[:, bass.ts(t, TILE)],
                    lhsT[:, bass.ts(c, CHUNK)],
                    start=True, stop=True)
                nc.scalar.activation(
                    s[:, bass.ts(c, CHUNK)], ps[:],
                    mybir.ActivationFunctionType.Identity,
                    bias=nsq_cols[:, t:t + 1], scale=2.0)
            nc.vector.tensor_add(s[:], s[:], nsq_bcast[:])

            # top-10 per row (descending S = ascending distance)
            m8 = small.tile([TILE, 8], F32)
            idx16 = small.tile([TILE, 16], U32)
            nc.vector.max(m8[:], s[:])
            nc.vector.max_index(idx16[:, 0:8], m8[:], s[:])
            nc.vector.match_replace(s[:], m8[:], s[:], -1e30)
            m8b = small.tile([TILE, 8], F32)
            nc.vector.max(m8b[:], s[:])
            nc.vector.max_index(idx16[:, 8:16], m8b[:], s[:])

            # gather neighbor coords: g[j, k*3+cc] = pos_T[idx[j,k], cc]
            # HW SWDGE consumes ONE offset per partition per DMA, so issue
            # one gather per neighbor slot k.
            g = small.tile([TILE, 30], F32)
            for k in range(K):
                nc.gpsimd.indirect_dma_start(
                    g[:, 3 * k:3 * k + 3], None,
                    posT_d[:],
                    bass.IndirectOffsetOnAxis(ap=idx16[:, k:k + 1], axis=0))

            # transpose to [30, 128]
            pt = psum_t.tile([30, TILE], F32)
            nc.tensor.transpose(pt[:], g[:], ident[:])
            gt = small.tile([30, TILE], F32)
            nc.scalar.activation(
                gt[:], pt[:], mybir.ActivationFunctionType.Copy)

            # out_tile[p, j] = sum_k WfusedT[k, p] * gt[k, j] + bias_eff[p]
            po = psum_o.tile([64, TILE], F32)
            nc.tensor.matmul(po[:], wf[:], gt[:], start=True, stop=True)
            ot = small.tile([64, TILE], F32)
            nc.scalar.activation(
                ot[:], po[:], mybir.ActivationFunctionType.Identity,
                bias=be[:])
            nc.sync.dma_start(out_d[:, bass.ts(t, TILE)], ot[:])


_PROGRAM = None
TRACE = False
LAST_EXEC_NS = None


def kernel(pos, w1, b1, w2, b2):
    global _PROGRAM
    pos = np.ascontiguousarray(np.asarray(pos, np.float32))
    Wfused, bias_eff = _fuse_weights(
        np.asarray(w1, np.float32), np.asarray(b1, np.float32),
        np.asarray(w2, np.float32), np.asarray(b2, np.float32))
    wfT = np.ascontiguousarray(Wfused.T)              # [30, 64]
    beC = np.ascontiguousarray(bias_eff[:, None])     # [64, 1]

    if _PROGRAM is None:
        _PROGRAM = _build_program()
    nc = _PROGRAM

    in_maps = []
    for b in range(B):
        in_maps.append({
            "pos": pos[b],
            "pos_t": np.ascontiguousarray(pos[b].T),
            "wfused_t": wfT,
            "bias_eff": beC,
        })
    import time
    t0 = time.perf_counter()
    res = run_bass_kernel_spmd(nc, in_maps, list(range(B)), trace=TRACE)
    t1 = time.perf_counter()
    global LAST_EXEC_NS
    LAST_EXEC_NS = res.exec_time_ns
    if LAST_EXEC_NS is None:
        LAST_EXEC_NS = int((t1 - t0) * 1e9)
    out = np.stack([res.results[b]["out"] for b in range(B)], 0)
    return out.astype(np.float32)
